# revision 19
# baseline (speedup 1.0000x reference)
"""Trainium2 Bass kernel for nn_DynamicShortConvolution.

Reference computation (per token t, channel d):
    h    = silu(x @ w1)                       # [T, H]
    flat = h @ w2 + b2                        # [T, D*W]
    k    = flat.reshape(T, D, W)
    out[t, d] = silu(sum_w k[t, d, w] * x[t - (W-1) + w, d])

Sharding: 8 cores, each one (batch, half-of-T) shard of 2048 tokens plus a
3-token left halo.  Per-core tensors are TRANSPOSED ([D, T], channels on
SBUF partitions) so the conv's token shift is a free-dim offset and both
matmuls consume/produce the natural layouts (no on-device transposes).

Schedule (engineered from the measured TRN2 cost model):
 - PE warm-up matmuls at t=0 cover the initial DMA latency (HAM stays warm).
 - mm1 accumulates dt-OUTER so each x d-tile is consumed as its DMA lands;
   x tiles stream on the sync HWDGE ring in consumption order, w2 follows.
 - mm2 evac per (dt, 1024-token pair): DVE does the two odd-tap fused
   (k+b)*x stt ops (PSUM 1x mode) plus the two even-tap bf16 2x-mode
   multiplies; ACT does the two even-tap biased PSUM evacs + final silu;
   the 3 tap-sum adds run as SBUF->SBUF accumulate-DMAs (SWDGE CCE) in
   mode 'v2', or on GpSimd/DVE in mode 'v1'.
"""

import os

import numpy as np

# Problem constants (hardcoded per harness contract).
B, T, D, H, W = 4, 4096, 2048, 256, 4
HALO = W - 1
N_CORES = 8
TOK = (B * T) // N_CORES  # tokens per core = 2048
N_DT = D // 128           # 16 d-tiles
N_HC = H // 128           # 2 h-tiles
XSTRIDE = TOK + HALO + 1  # 2052, even keeps per-dtile 4B alignment
CH = 512                  # mm1 token chunk (one PSUM bank of fp32)
P = 1024                  # mm2 token pair width (two banks per tap)
N_WARM = 8                # PE warm-up matmuls (~3.4us at cold clock)


def _build_nc(mode="v2", out_f32=False):
    import concourse.bass as bass  # noqa: F401
    import concourse.bacc as bacc
    import concourse.mybir as mybir
    import concourse.tile as tile
    from concourse.ap import AP as BassAP

    f32 = mybir.dt.float32
    bf16 = mybir.dt.bfloat16
    AF = mybir.ActivationFunctionType
    ALU = mybir.AluOpType

    tok, h = TOK, H
    n_dt, n_hc = N_DT, N_HC

    nc = bacc.Bacc()

    # DRAM I/O (host-prepared layouts; see _prep_shards)
    xT = nc.declare_dram_parameter("xT", [n_dt, 128, XSTRIDE], bf16, isOutput=False)
    w1r = nc.declare_dram_parameter("w1r", [128, n_dt * h], bf16, isOutput=False)
    # w2r[dt, p, hc*512 + w*128 + i] = w2[hc*128+p, (dt*128+i)*W + w]
    w2r = nc.declare_dram_parameter("w2r", [n_dt, 128, n_hc * W * 128], bf16,
                                    isOutput=False)
    # b2r[p, dt*W + w] = b2[(dt*128+p)*W + w]
    b2r = nc.declare_dram_parameter("b2r", [128, n_dt * W], f32, isOutput=False)
    out_dt = f32 if out_f32 else bf16
    outT = nc.declare_dram_parameter("outT", [n_dt, 128, tok], out_dt, isOutput=True)

    with tile.TileContext(nc) as tc:
        with (
            tc.tile_pool(name="resident", bufs=1) as rpool,
            tc.tile_pool(name="work", bufs=3) as wpool,
            tc.tile_pool(name="psum", bufs=4, space="PSUM") as ppool,
        ):
            # ---- PE warm-up (covers initial DMA latency, warms HAM) ----
            warm_sb = rpool.tile([128, 640], bf16, tag="warm")
            nc.vector.memset(warm_sb[:], 0.0)
            for _ in range(N_WARM):
                wt = ppool.tile([128, P], f32, tag="ps")
                nc.tensor.matmul(wt[:, :CH], warm_sb[:, :128],
                                 warm_sb[:, 128:640], start=True, stop=True)

            # ---- resident tiles ----
            xT_sb = rpool.tile([128, n_dt * XSTRIDE], bf16, tag="xT")
            w1_sb = rpool.tile([128, n_dt * h], bf16, tag="w1")
            w2_sb = rpool.tile([128, n_dt * n_hc * W * 128], bf16, tag="w2")
            b2_sb = rpool.tile([128, n_dt * W], f32, tag="b2")
            hT_sb = rpool.tile([128, n_hc * tok], bf16, tag="hT")

            # DMA order = consumption order: w1, then x d-tiles, then w2.
            # b2 rides the scalar (ACT) HWDGE ring so it never queues x.
            nc.scalar.dma_start(b2_sb[:], b2r[:])
            nc.sync.dma_start(w1_sb[:], w1r[:])
            for dt in range(n_dt):
                nc.sync.dma_start(
                    xT_sb[:, dt * XSTRIDE:(dt + 1) * XSTRIDE], xT[dt])
            for dt in range(n_dt):
                nc.sync.dma_start(
                    w2_sb[:, dt * 1024:(dt + 1) * 1024], w2r[dt])

            def x_slice(dt, col, n):
                return xT_sb[:, dt * XSTRIDE + col: dt * XSTRIDE + col + n]

            # ---- mm1: hT = silu(w1.T @ xT), dt-OUTER accumulation ----
            # 8 groups (hc, tc) live in 4 [128,1024] psum tiles (2 banks
            # each, one bank per group) so each arriving x d-tile is
            # consumed immediately.
            ph = [ppool.tile([128, P], f32, tag="ps", name=f"ph{i}")
                  for i in range(4)]

            def ph_half(g):
                c = (g % 2) * CH
                return ph[g // 2][:, c:c + CH]

            for dt in range(n_dt):
                for hc in range(n_hc):
                    for tcb in range(4):
                        g = hc * 4 + tcb
                        nc.tensor.matmul(
                            ph_half(g),
                            w1_sb[:, dt * h + hc * 128: dt * h + hc * 128 + 128],
                            x_slice(dt, HALO + tcb * CH, CH),
                            start=(dt == 0), stop=(dt == n_dt - 1))
            for g in range(8):
                hc, tcb = g // 4, g % 4
                nc.scalar.activation(
                    hT_sb[:, hc * tok + tcb * CH: hc * tok + (tcb + 1) * CH],
                    ph_half(g), AF.Silu)

            # ---- mm2 + conv + silu, per (dt, 1024-token pair) ----
            # Two iterations form a "duo" with tap sums split across two
            # tiles:  me = [e.m0 | e.m2 | o.m0 | o.m2]   (survives to silu)
            #         mo = [e.m1 | e.m3 | o.m1 | o.m3]   (freed after AB)
            # Tap-sum = two SBUF->SBUF accumulate-DMAs per duo (SWDGE CCE,
            # half the descriptor-gen of per-iteration DMAs — desc-gen on
            # GpSimd contends with DVE SBUF reads):
            #   AB: me += mo (contiguous 4P)   C: me.[m0s] += me.[m2s]
            # C lags two iterations, silu + out-DMA lag two more.
            stageC = []   # [(me, [(dt, j0), (dt, j0)])] awaiting C
            stageS = []   # same, C emitted, awaiting silu

            def emit_c():
                item = stageC.pop(0)
                me = item[0]
                sl = me[:, 0:P]
                dst = BassAP(tensor=sl.tensor, offset=sl.offset,
                             ap=[list(sl.ap[0]), [2 * P, 2], [1, P]])
                sl2 = me[:, P:2 * P]
                src = BassAP(tensor=sl2.tensor, offset=sl2.offset,
                             ap=[list(sl2.ap[0]), [2 * P, 2], [1, P]])
                nc.gpsimd.dma_start(dst, src, accum_op=ALU.add)
                stageS.append(item)

            def emit_silu():
                me, locs = stageS.pop(0)
                for half, (pdt, pj0) in enumerate(locs):
                    ot = wpool.tile([128, P], out_dt, tag="ot", name="ot")
                    nc.scalar.activation(
                        ot[:], me[:, half * 2 * P: half * 2 * P + P], AF.Silu)
                    nc.sync.dma_start(outT[pdt, :, pj0:pj0 + P], ot[:])

            me, mo, duo_locs = None, None, []
            for it in range(n_dt * (tok // P)):
                dt, pi = it // 2, it % 2
                j0 = pi * P
                kw = [ppool.tile([128, P], f32, tag="ps", name=f"kw{w}")
                      for w in range(W)]
                for w in range(W):
                    wcol = dt * (n_hc * W * 128) + w * 128
                    for hc in range(n_hc):
                        for tcj in range(2):
                            nc.tensor.matmul(
                                kw[w][:, tcj * CH:(tcj + 1) * CH],
                                w2_sb[:, wcol + hc * W * 128:
                                      wcol + hc * W * 128 + 128],
                                hT_sb[:, hc * tok + j0 + tcj * CH:
                                      hc * tok + j0 + (tcj + 1) * CH],
                                start=(hc == 0), stop=(hc == n_hc - 1))
                if it % 2 == 0:
                    me = wpool.tile([128, 4 * P], bf16, tag="me", bufs=5)
                    mo = wpool.tile([128, 4 * P], bf16, tag="mo", bufs=2)
                    duo_locs = []
                    if len(stageC) >= 2:
                        emit_c()  # C for the duo before last, deps met
                mb = (it % 2) * 2 * P
                duo_locs.append((dt, j0))
                bias = [b2_sb[:, dt * W + w: dt * W + w + 1]
                        for w in range(W)]
                kb = wpool.tile([128, 2 * P], bf16, tag="kb")
                # odd taps (2B-misaligned x windows): DVE fused
                # evac+bias+mul, PSUM 1x mode.  even taps: ACT biased
                # evac, then DVE bf16 2x-mode mul (x windows 4B-aligned)
                nc.vector.scalar_tensor_tensor(
                    mo[:, mb:mb + P], kw[1][:], bias[1],
                    x_slice(dt, j0 + 1, P), op0=ALU.add, op1=ALU.mult)
                nc.scalar.activation(kb[:, 0:P], kw[0][:], AF.Identity,
                                     bias=bias[0])
                nc.vector.tensor_mul(me[:, mb:mb + P], kb[:, 0:P],
                                     x_slice(dt, j0 + 0, P))
                nc.vector.scalar_tensor_tensor(
                    mo[:, mb + P:mb + 2 * P], kw[3][:], bias[3],
                    x_slice(dt, j0 + 3, P), op0=ALU.add, op1=ALU.mult)
                nc.scalar.activation(kb[:, P:2 * P], kw[2][:], AF.Identity,
                                     bias=bias[2])
                nc.vector.tensor_mul(me[:, mb + P:mb + 2 * P],
                                     kb[:, P:2 * P], x_slice(dt, j0 + 2, P))
                if len(stageS) >= 3:
                    emit_silu()
                if it % 2 == 1:
                    # AB for this duo: me += mo (two 2P accums: the CCE
                    # path faults on runs wider than 2048 elements)
                    nc.gpsimd.dma_start(me[:, 0:2 * P], mo[:, 0:2 * P],
                                        accum_op=ALU.add)
                    nc.gpsimd.dma_start(me[:, 2 * P:4 * P], mo[:, 2 * P:4 * P],
                                        accum_op=ALU.add)
                    stageC.append((me, duo_locs))
            while stageC:
                emit_c()
            while stageS:
                emit_silu()
    nc.compile()
    return nc


def _prep_shards(x, w1, w2, b2):
    """Host-side shard prep. Returns list of per-core in_maps."""
    import ml_dtypes
    bf16 = ml_dtypes.bfloat16

    n_dt, n_hc, tok = N_DT, N_HC, TOK
    b, t, d = x.shape
    shards_per_batch = (b * t // tok) // b

    w1_r = np.ascontiguousarray(
        w1.reshape(n_dt, 128, H).transpose(1, 0, 2).reshape(128, n_dt * H)
    ).astype(bf16)
    w2_r = np.ascontiguousarray(
        w2.reshape(n_hc, 128, n_dt, 128, W).transpose(2, 1, 0, 4, 3)
        .reshape(n_dt, 128, n_hc * W * 128)).astype(bf16)
    b2_r = np.ascontiguousarray(
        b2.reshape(n_dt, 128, W).transpose(1, 0, 2).reshape(128, n_dt * W)
    ).astype(np.float32)

    in_maps = []
    for core in range(N_CORES):
        bi, half = divmod(core, shards_per_batch)
        t0 = half * tok
        xh = np.zeros((tok + HALO, d), np.float32)
        lo = max(t0 - HALO, 0)
        xh[HALO - (t0 - lo):] = x[bi, lo: t0 + tok]
        xTc = np.zeros((n_dt, 128, XSTRIDE), bf16)
        xTc[:, :, : tok + HALO] = (
            xh.T.astype(bf16).reshape(n_dt, 128, tok + HALO))
        in_maps.append({"xT": xTc, "w1r": w1_r, "w2r": w2_r, "b2r": b2_r})
    return in_maps


_NC_CACHE = {}


def kernel(x, w1, w2, b2, trace=False):
    from concourse.bass_utils import run_bass_kernel_spmd

    mode = os.environ.get("KMODE", "v2")
    if mode not in _NC_CACHE:
        _NC_CACHE[mode] = _build_nc(mode=mode, out_f32=False)
    nc = _NC_CACHE[mode]

    in_maps = _prep_shards(
        np.asarray(x, np.float32), np.asarray(w1, np.float32),
        np.asarray(w2, np.float32), np.asarray(b2, np.float32))

    res = run_bass_kernel_spmd(nc, in_maps, core_ids=list(range(N_CORES)),
                               trace=trace)
    kernel.last_result = res

    shards_per_batch = (B * T // TOK) // B
    out = np.empty((B, T, D), np.float32)
    for core in range(N_CORES):
        bi, half = divmod(core, shards_per_batch)
        oT = res.results[core]["outT"]  # [n_dt, 128, tok]
        out[bi, half * TOK:(half + 1) * TOK] = (
            oT.reshape(D, TOK).T.astype(np.float32))
    return out


# revision 21
# speedup vs baseline: 1.0622x; 1.0622x over previous
"""Trainium2 Bass kernel for nn_DynamicShortConvolution.

Reference computation (per token t, channel d):
    h    = silu(x @ w1)                       # [T, H]
    flat = h @ w2 + b2                        # [T, D*W]
    k    = flat.reshape(T, D, W)
    out[t, d] = silu(sum_w k[t, d, w] * x[t - (W-1) + w, d])

Sharding: 8 cores, each one (batch, half-of-T) shard of 2048 tokens plus a
3-token left halo.  Per-core tensors are TRANSPOSED ([D, T], channels on
SBUF partitions) so the conv's token shift is a free-dim offset and both
matmuls consume/produce the natural layouts (no on-device transposes).

Schedule (engineered from the measured TRN2 cost model):
 - PE warm-up matmuls at t=0 cover the initial DMA latency (HAM stays warm).
 - mm1 accumulates dt-OUTER so each x d-tile is consumed as its DMA lands;
   x tiles stream on the sync HWDGE ring in consumption order, w2 follows.
 - mm2 evac per (dt, 1024-token pair): DVE does the two odd-tap fused
   (k+b)*x stt ops (PSUM 1x mode) plus the two even-tap bf16 2x-mode
   multiplies; ACT does the two even-tap biased PSUM evacs + final silu;
   the 3 tap-sum adds run as SBUF->SBUF accumulate-DMAs (SWDGE CCE) in
   mode 'v2', or on GpSimd/DVE in mode 'v1'.
"""

import os

import numpy as np

# Problem constants (hardcoded per harness contract).
B, T, D, H, W = 4, 4096, 2048, 256, 4
HALO = W - 1
N_CORES = 8
TOK = (B * T) // N_CORES  # tokens per core = 2048
N_DT = D // 128           # 16 d-tiles
N_HC = H // 128           # 2 h-tiles
XSTRIDE = TOK + HALO + 1  # 2052, even keeps per-dtile 4B alignment
CH = 512                  # mm1 token chunk (one PSUM bank of fp32)
P = 1024                  # mm2 token pair width (two banks per tap)
N_WARM = 8                # PE warm-up matmuls (~3.4us at cold clock)


def _build_nc(mode="v2", out_f32=False):
    import concourse.bass as bass  # noqa: F401
    import concourse.bacc as bacc
    import concourse.mybir as mybir
    import concourse.tile as tile
    from concourse.ap import AP as BassAP

    f32 = mybir.dt.float32
    bf16 = mybir.dt.bfloat16
    AF = mybir.ActivationFunctionType
    ALU = mybir.AluOpType

    tok, h = TOK, H
    n_dt, n_hc = N_DT, N_HC

    nc = bacc.Bacc()

    # DRAM I/O (host-prepared layouts; see _prep_shards)
    xT = nc.declare_dram_parameter("xT", [n_dt, 128, XSTRIDE], bf16, isOutput=False)
    w1r = nc.declare_dram_parameter("w1r", [128, n_dt * h], bf16, isOutput=False)
    # w2r[dt, p, hc*512 + w*128 + i] = w2[hc*128+p, (dt*128+i)*W + w]
    w2r = nc.declare_dram_parameter("w2r", [n_dt, 128, n_hc * W * 128], bf16,
                                    isOutput=False)
    # b2r[p, dt*W + w] = b2[(dt*128+p)*W + w]
    b2r = nc.declare_dram_parameter("b2r", [128, n_dt * W], f32, isOutput=False)
    out_dt = f32 if out_f32 else bf16
    outT = nc.declare_dram_parameter("outT", [n_dt, 128, tok], out_dt, isOutput=True)

    with tile.TileContext(nc) as tc:
        with (
            tc.tile_pool(name="resident", bufs=1) as rpool,
            tc.tile_pool(name="work", bufs=3) as wpool,
            tc.tile_pool(name="psum", bufs=4, space="PSUM") as ppool,
        ):
            # ---- PE warm-up (covers initial DMA latency, warms HAM) ----
            warm_sb = rpool.tile([128, 640], bf16, tag="warm")
            nc.vector.memset(warm_sb[:], 0.0)
            for _ in range(N_WARM):
                wt = ppool.tile([128, P], f32, tag="ps")
                nc.tensor.matmul(wt[:, :CH], warm_sb[:, :128],
                                 warm_sb[:, 128:640], start=True, stop=True)

            # ---- resident tiles ----
            xT_sb = rpool.tile([128, n_dt * XSTRIDE], bf16, tag="xT")
            w1_sb = rpool.tile([128, n_dt * h], bf16, tag="w1")
            w2_sb = rpool.tile([128, n_dt * n_hc * W * 128], bf16, tag="w2")
            b2_sb = rpool.tile([128, n_dt * W], f32, tag="b2")
            hT_sb = rpool.tile([128, n_hc * tok], bf16, tag="hT")

            # DMA order = consumption order: w1, then x d-tiles, then w2.
            # b2 rides the scalar (ACT) HWDGE ring so it never queues x.
            nc.scalar.dma_start(b2_sb[:], b2r[:])
            nc.sync.dma_start(w1_sb[:], w1r[:])
            for dt in range(n_dt):
                nc.sync.dma_start(
                    xT_sb[:, dt * XSTRIDE:(dt + 1) * XSTRIDE], xT[dt])
            for dt in range(n_dt):
                nc.sync.dma_start(
                    w2_sb[:, dt * 1024:(dt + 1) * 1024], w2r[dt])

            def x_slice(dt, col, n):
                return xT_sb[:, dt * XSTRIDE + col: dt * XSTRIDE + col + n]

            # ---- mm1: hT = silu(w1.T @ xT), dt-OUTER accumulation ----
            # 8 groups (hc, tc) live in 4 [128,1024] psum tiles (2 banks
            # each, one bank per group) so each arriving x d-tile is
            # consumed immediately.
            ph = [ppool.tile([128, P], f32, tag="ps", name=f"ph{i}")
                  for i in range(4)]

            def ph_half(g):
                c = (g % 2) * CH
                return ph[g // 2][:, c:c + CH]

            for dt in range(n_dt):
                for hc in range(n_hc):
                    for tcb in range(4):
                        g = hc * 4 + tcb
                        nc.tensor.matmul(
                            ph_half(g),
                            w1_sb[:, dt * h + hc * 128: dt * h + hc * 128 + 128],
                            x_slice(dt, HALO + tcb * CH, CH),
                            start=(dt == 0), stop=(dt == n_dt - 1))
            for g in range(8):
                hc, tcb = g // 4, g % 4
                nc.scalar.activation(
                    hT_sb[:, hc * tok + tcb * CH: hc * tok + (tcb + 1) * CH],
                    ph_half(g), AF.Silu)

            # ---- mm2 + conv + silu, per (dt, 1024-token pair) ----
            # Two iterations form a "duo" with tap sums split across two
            # tiles:  me = [e.m0 | e.m2 | o.m0 | o.m2]   (survives to silu)
            #         mo = [e.m1 | e.m3 | o.m1 | o.m3]   (freed after AB)
            # Tap-sum = two SBUF->SBUF accumulate-DMAs per duo (SWDGE CCE,
            # half the descriptor-gen of per-iteration DMAs — desc-gen on
            # GpSimd contends with DVE SBUF reads):
            #   AB: me += mo (contiguous 4P)   C: me.[m0s] += me.[m2s]
            # C lags two iterations, silu + out-DMA lag two more.
            stageC = []   # [(me, [(dt, j0), (dt, j0)])] awaiting C
            stageH = []   # [(me, half, dt, j0)] C emitted, awaiting silu

            def emit_c():
                me, locs = stageC.pop(0)
                sl = me[:, 0:P]
                dst = BassAP(tensor=sl.tensor, offset=sl.offset,
                             ap=[list(sl.ap[0]), [2 * P, 2], [1, P]])
                sl2 = me[:, P:2 * P]
                src = BassAP(tensor=sl2.tensor, offset=sl2.offset,
                             ap=[list(sl2.ap[0]), [2 * P, 2], [1, P]])
                nc.gpsimd.dma_start(dst, src, accum_op=ALU.add)
                for half, (pdt, pj0) in enumerate(locs):
                    stageH.append((me, half, pdt, pj0))

            def emit_silu():
                me, half, pdt, pj0 = stageH.pop(0)
                ot = wpool.tile([128, P], out_dt, tag="ot", name="ot", bufs=4)
                nc.scalar.activation(
                    ot[:], me[:, half * 2 * P: half * 2 * P + P], AF.Silu)
                nc.sync.dma_start(outT[pdt, :, pj0:pj0 + P], ot[:])

            me, mo, duo_locs = None, None, []
            for it in range(n_dt * (tok // P)):
                dt, pi = it // 2, it % 2
                j0 = pi * P
                kw = [ppool.tile([128, P], f32, tag="ps", name=f"kw{w}")
                      for w in range(W)]
                for w in range(W):
                    wcol = dt * (n_hc * W * 128) + w * 128
                    for hc in range(n_hc):
                        for tcj in range(2):
                            nc.tensor.matmul(
                                kw[w][:, tcj * CH:(tcj + 1) * CH],
                                w2_sb[:, wcol + hc * W * 128:
                                      wcol + hc * W * 128 + 128],
                                hT_sb[:, hc * tok + j0 + tcj * CH:
                                      hc * tok + j0 + (tcj + 1) * CH],
                                start=(hc == 0), stop=(hc == n_hc - 1))
                if it % 2 == 0:
                    me = wpool.tile([128, 4 * P], bf16, tag="me", bufs=5)
                    mo = wpool.tile([128, 4 * P], bf16, tag="mo", bufs=2)
                    duo_locs = []
                    if len(stageC) >= 2:
                        emit_c()  # C for the duo before last, deps met
                mb = (it % 2) * 2 * P
                duo_locs.append((dt, j0))
                bias = [b2_sb[:, dt * W + w: dt * W + w + 1]
                        for w in range(W)]
                kb = wpool.tile([128, 2 * P], bf16, tag="kb", bufs=4)
                # odd taps (2B-misaligned x windows): DVE fused
                # evac+bias+mul, PSUM 1x mode.  even taps: ACT biased
                # evac, then DVE bf16 2x-mode mul (x windows 4B-aligned)
                nc.vector.scalar_tensor_tensor(
                    mo[:, mb:mb + P], kw[1][:], bias[1],
                    x_slice(dt, j0 + 1, P), op0=ALU.add, op1=ALU.mult)
                nc.scalar.activation(kb[:, 0:P], kw[0][:], AF.Identity,
                                     bias=bias[0])
                nc.vector.tensor_mul(me[:, mb:mb + P], kb[:, 0:P],
                                     x_slice(dt, j0 + 0, P))
                nc.vector.scalar_tensor_tensor(
                    mo[:, mb + P:mb + 2 * P], kw[3][:], bias[3],
                    x_slice(dt, j0 + 3, P), op0=ALU.add, op1=ALU.mult)
                nc.scalar.activation(kb[:, P:2 * P], kw[2][:], AF.Identity,
                                     bias=bias[2])
                nc.vector.tensor_mul(me[:, mb + P:mb + 2 * P],
                                     kb[:, P:2 * P], x_slice(dt, j0 + 2, P))
                if len(stageH) >= 3:
                    emit_silu()
                if it % 2 == 1:
                    # AB for this duo: me += mo (two 2P accums: the CCE
                    # path faults on runs wider than 2048 elements)
                    nc.gpsimd.dma_start(me[:, 0:2 * P], mo[:, 0:2 * P],
                                        accum_op=ALU.add)
                    nc.gpsimd.dma_start(me[:, 2 * P:4 * P], mo[:, 2 * P:4 * P],
                                        accum_op=ALU.add)
                    stageC.append((me, duo_locs))
            while stageC:
                emit_c()
            while stageH:
                emit_silu()
    nc.compile()
    return nc


def _prep_shards(x, w1, w2, b2):
    """Host-side shard prep. Returns list of per-core in_maps."""
    import ml_dtypes
    bf16 = ml_dtypes.bfloat16

    n_dt, n_hc, tok = N_DT, N_HC, TOK
    b, t, d = x.shape
    shards_per_batch = (b * t // tok) // b

    w1_r = np.ascontiguousarray(
        w1.reshape(n_dt, 128, H).transpose(1, 0, 2).reshape(128, n_dt * H)
    ).astype(bf16)
    w2_r = np.ascontiguousarray(
        w2.reshape(n_hc, 128, n_dt, 128, W).transpose(2, 1, 0, 4, 3)
        .reshape(n_dt, 128, n_hc * W * 128)).astype(bf16)
    b2_r = np.ascontiguousarray(
        b2.reshape(n_dt, 128, W).transpose(1, 0, 2).reshape(128, n_dt * W)
    ).astype(np.float32)

    in_maps = []
    for core in range(N_CORES):
        bi, half = divmod(core, shards_per_batch)
        t0 = half * tok
        xh = np.zeros((tok + HALO, d), np.float32)
        lo = max(t0 - HALO, 0)
        xh[HALO - (t0 - lo):] = x[bi, lo: t0 + tok]
        xTc = np.zeros((n_dt, 128, XSTRIDE), bf16)
        xTc[:, :, : tok + HALO] = (
            xh.T.astype(bf16).reshape(n_dt, 128, tok + HALO))
        in_maps.append({"xT": xTc, "w1r": w1_r, "w2r": w2_r, "b2r": b2_r})
    return in_maps


_NC_CACHE = {}


def kernel(x, w1, w2, b2, trace=False):
    from concourse.bass_utils import run_bass_kernel_spmd

    mode = os.environ.get("KMODE", "v2")
    if mode not in _NC_CACHE:
        _NC_CACHE[mode] = _build_nc(mode=mode, out_f32=False)
    nc = _NC_CACHE[mode]

    in_maps = _prep_shards(
        np.asarray(x, np.float32), np.asarray(w1, np.float32),
        np.asarray(w2, np.float32), np.asarray(b2, np.float32))

    res = run_bass_kernel_spmd(nc, in_maps, core_ids=list(range(N_CORES)),
                               trace=trace)
    kernel.last_result = res

    shards_per_batch = (B * T // TOK) // B
    out = np.empty((B, T, D), np.float32)
    for core in range(N_CORES):
        bi, half = divmod(core, shards_per_batch)
        oT = res.results[core]["outT"]  # [n_dt, 128, tok]
        out[bi, half * TOK:(half + 1) * TOK] = (
            oT.reshape(D, TOK).T.astype(np.float32))
    return out


# revision 23
# speedup vs baseline: 1.1162x; 1.0509x over previous
"""Trainium2 Bass kernel for nn_DynamicShortConvolution.

Reference computation (per token t, channel d):
    h    = silu(x @ w1)                       # [T, H]
    flat = h @ w2 + b2                        # [T, D*W]
    k    = flat.reshape(T, D, W)
    out[t, d] = silu(sum_w k[t, d, w] * x[t - (W-1) + w, d])

Sharding: 8 cores, each one (batch, half-of-T) shard of 2048 tokens plus a
3-token left halo.  Per-core tensors are TRANSPOSED ([D, T], channels on
SBUF partitions) so the conv's token shift is a free-dim offset and both
matmuls consume/produce the natural layouts (no on-device transposes).

Schedule (engineered from the measured TRN2 cost model):
 - PE warm-up matmuls at t=0 cover the initial DMA latency (HAM stays warm).
 - mm1 accumulates dt-OUTER so each x d-tile is consumed as its DMA lands;
   x tiles stream on the sync HWDGE ring in consumption order, w2 follows.
 - mm2 evac per (dt, 1024-token pair): DVE does the two odd-tap fused
   (k+b)*x stt ops (PSUM 1x mode) plus the two even-tap bf16 2x-mode
   multiplies; ACT does the two even-tap biased PSUM evacs + final silu;
   the 3 tap-sum adds run as SBUF->SBUF accumulate-DMAs (SWDGE CCE) in
   mode 'v2', or on GpSimd/DVE in mode 'v1'.
"""

import os

import numpy as np

# Problem constants (hardcoded per harness contract).
B, T, D, H, W = 4, 4096, 2048, 256, 4
HALO = W - 1
N_CORES = 8
TOK = (B * T) // N_CORES  # tokens per core = 2048
N_DT = D // 128           # 16 d-tiles
N_HC = H // 128           # 2 h-tiles
XSTRIDE = TOK + HALO + 1  # 2052, even keeps per-dtile 4B alignment
CH = 512                  # mm1 token chunk (one PSUM bank of fp32)
P = 1024                  # mm2 token pair width (two banks per tap)
N_WARM = 8                # PE warm-up matmuls (~3.4us at cold clock)


def _build_nc(mode="v2", out_f32=False):
    import concourse.bass as bass  # noqa: F401
    import concourse.bacc as bacc
    import concourse.mybir as mybir
    import concourse.tile as tile
    from concourse.ap import AP as BassAP

    f32 = mybir.dt.float32
    bf16 = mybir.dt.bfloat16
    AF = mybir.ActivationFunctionType
    ALU = mybir.AluOpType

    tok, h = TOK, H
    n_dt, n_hc = N_DT, N_HC

    nc = bacc.Bacc()

    # DRAM I/O (host-prepared layouts; see _prep_shards)
    xT = nc.declare_dram_parameter("xT", [n_dt, 128, XSTRIDE], bf16, isOutput=False)
    w1r = nc.declare_dram_parameter("w1r", [128, n_dt * h], bf16, isOutput=False)
    # w2r[dt, p, hc*512 + w*128 + i] = w2[hc*128+p, (dt*128+i)*W + w]
    w2r = nc.declare_dram_parameter("w2r", [n_dt, 128, n_hc * W * 128], bf16,
                                    isOutput=False)
    # b2r[p, dt*W + w] = b2[(dt*128+p)*W + w]
    b2r = nc.declare_dram_parameter("b2r", [128, n_dt * W], f32, isOutput=False)
    out_dt = f32 if out_f32 else bf16
    outT = nc.declare_dram_parameter("outT", [n_dt, 128, tok], out_dt, isOutput=True)

    with tile.TileContext(nc) as tc:
        with (
            tc.tile_pool(name="resident", bufs=1) as rpool,
            tc.tile_pool(name="work", bufs=3) as wpool,
            tc.tile_pool(name="psum", bufs=4, space="PSUM") as ppool,
        ):
            # ---- PE warm-up (covers initial DMA latency, warms HAM) ----
            warm_sb = rpool.tile([128, 640], bf16, tag="warm")
            nc.vector.memset(warm_sb[:], 0.0)
            for _ in range(N_WARM):
                wt = ppool.tile([128, P], f32, tag="ps")
                nc.tensor.matmul(wt[:, :CH], warm_sb[:, :128],
                                 warm_sb[:, 128:640], start=True, stop=True)

            # ---- resident tiles ----
            xT_sb = rpool.tile([128, n_dt * XSTRIDE], bf16, tag="xT")
            w1_sb = rpool.tile([128, n_dt * h], bf16, tag="w1")
            w2_sb = rpool.tile([128, n_dt * n_hc * W * 128], bf16, tag="w2")
            b2_sb = rpool.tile([128, n_dt * W], f32, tag="b2")
            hT_sb = rpool.tile([128, n_hc * tok], bf16, tag="hT")

            # DMA order = consumption order: w1, then x d-tiles, then w2.
            # b2 rides the scalar (ACT) HWDGE ring so it never queues x.
            nc.scalar.dma_start(b2_sb[:], b2r[:])
            nc.sync.dma_start(w1_sb[:], w1r[:])
            for dt in range(n_dt):
                nc.sync.dma_start(
                    xT_sb[:, dt * XSTRIDE:(dt + 1) * XSTRIDE], xT[dt])
            for dt in range(n_dt):
                nc.sync.dma_start(
                    w2_sb[:, dt * 1024:(dt + 1) * 1024], w2r[dt])

            def x_slice(dt, col, n):
                return xT_sb[:, dt * XSTRIDE + col: dt * XSTRIDE + col + n]

            # ---- mm1: hT = silu(w1.T @ xT), dt-OUTER accumulation ----
            # 8 groups (hc, tc) live in 4 [128,1024] psum tiles (2 banks
            # each, one bank per group) so each arriving x d-tile is
            # consumed immediately.
            ph = [ppool.tile([128, P], f32, tag="ps", name=f"ph{i}")
                  for i in range(4)]

            def ph_half(g):
                c = (g % 2) * CH
                return ph[g // 2][:, c:c + CH]

            for dt in range(n_dt):
                for hc in range(n_hc):
                    for tcb in range(4):
                        g = hc * 4 + tcb
                        nc.tensor.matmul(
                            ph_half(g),
                            w1_sb[:, dt * h + hc * 128: dt * h + hc * 128 + 128],
                            x_slice(dt, HALO + tcb * CH, CH),
                            start=(dt == 0), stop=(dt == n_dt - 1))
            for g in range(8):
                hc, tcb = g // 4, g % 4
                nc.scalar.activation(
                    hT_sb[:, hc * tok + tcb * CH: hc * tok + (tcb + 1) * CH],
                    ph_half(g), AF.Silu)

            # ---- mm2 + conv + silu, per (dt, 1024-token pair) ----
            # Two iterations form a "duo" with tap sums split across two
            # tiles:  me = [e.m0 | e.m2 | o.m0 | o.m2]   (survives to silu)
            #         mo = [e.m1 | e.m3 | o.m1 | o.m3]   (freed after AB)
            # Tap-sum = two SBUF->SBUF accumulate-DMAs per duo (SWDGE CCE,
            # half the descriptor-gen of per-iteration DMAs — desc-gen on
            # GpSimd contends with DVE SBUF reads):
            #   AB: me += mo (contiguous 4P)   C: me.[m0s] += me.[m2s]
            # C lags two iterations, silu + out-DMA lag two more.
            stageC = []   # [(me, [(dt, j0), (dt, j0)])] awaiting C
            stageH = []   # [(me, half, dt, j0)] C emitted, awaiting silu

            def emit_c():
                me, locs = stageC.pop(0)
                sl = me[:, 0:P]
                dst = BassAP(tensor=sl.tensor, offset=sl.offset,
                             ap=[list(sl.ap[0]), [2 * P, 2], [1, P]])
                sl2 = me[:, P:2 * P]
                src = BassAP(tensor=sl2.tensor, offset=sl2.offset,
                             ap=[list(sl2.ap[0]), [2 * P, 2], [1, P]])
                nc.gpsimd.dma_start(dst, src, accum_op=ALU.add)
                for half, (pdt, pj0) in enumerate(locs):
                    stageH.append((me, half, pdt, pj0))

            def emit_silu():
                me, half, pdt, pj0 = stageH.pop(0)
                ot = wpool.tile([128, P], out_dt, tag="ot", name="ot", bufs=4)
                nc.scalar.activation(
                    ot[:], me[:, half * 2 * P: half * 2 * P + P], AF.Silu)
                nc.sync.dma_start(outT[pdt, :, pj0:pj0 + P], ot[:])

            me, mo, duo_locs = None, None, []
            for it in range(n_dt * (tok // P)):
                dt, pi = it // 2, it % 2
                j0 = pi * P
                kw = [ppool.tile([128, P], f32, tag="ps", name=f"kw{w}")
                      for w in range(W)]
                for w in range(W):
                    wcol = dt * (n_hc * W * 128) + w * 128
                    for hc in range(n_hc):
                        for tcj in range(2):
                            nc.tensor.matmul(
                                kw[w][:, tcj * CH:(tcj + 1) * CH],
                                w2_sb[:, wcol + hc * W * 128:
                                      wcol + hc * W * 128 + 128],
                                hT_sb[:, hc * tok + j0 + tcj * CH:
                                      hc * tok + j0 + (tcj + 1) * CH],
                                start=(hc == 0), stop=(hc == n_hc - 1))
                if it % 2 == 0:
                    me = wpool.tile([128, 4 * P], bf16, tag="me", bufs=4)
                    mo = wpool.tile([128, 4 * P], bf16, tag="mo", bufs=2)
                    duo_locs = []
                    if len(stageC) >= 2:
                        emit_c()  # C for the duo before last, deps met
                mb = (it % 2) * 2 * P
                duo_locs.append((dt, j0))
                bias = [b2_sb[:, dt * W + w: dt * W + w + 1]
                        for w in range(W)]
                kb = wpool.tile([128, 2 * P], bf16, tag="kb", bufs=4)
                # odd taps (2B-misaligned x windows): DVE fused
                # evac+bias+mul, PSUM 1x mode.  even taps: ACT biased
                # evac, then DVE bf16 2x-mode mul (x windows 4B-aligned)
                nc.vector.scalar_tensor_tensor(
                    mo[:, mb:mb + P], kw[1][:], bias[1],
                    x_slice(dt, j0 + 1, P), op0=ALU.add, op1=ALU.mult)
                nc.scalar.activation(kb[:, 0:P], kw[0][:], AF.Identity,
                                     bias=bias[0])
                nc.vector.tensor_mul(me[:, mb:mb + P], kb[:, 0:P],
                                     x_slice(dt, j0 + 0, P))
                nc.vector.scalar_tensor_tensor(
                    mo[:, mb + P:mb + 2 * P], kw[3][:], bias[3],
                    x_slice(dt, j0 + 3, P), op0=ALU.add, op1=ALU.mult)
                nc.scalar.activation(kb[:, P:2 * P], kw[2][:], AF.Identity,
                                     bias=bias[2])
                nc.vector.tensor_mul(me[:, mb + P:mb + 2 * P],
                                     kb[:, P:2 * P], x_slice(dt, j0 + 2, P))
                if len(stageH) >= 3:
                    emit_silu()
                n_it = n_dt * (tok // P)
                if it >= n_it - 4:
                    # tail iterations: engines drain soon, so sum the taps
                    # inline on DVE/ACT instead of the long DMA-accum chain
                    g2 = wpool.tile([128, 2 * P], bf16, tag="g2", bufs=2)
                    nc.vector.tensor_add(g2[:, 0:P], me[:, mb:mb + P],
                                         mo[:, mb:mb + P])
                    nc.vector.tensor_add(g2[:, P:2 * P],
                                         me[:, mb + P:mb + 2 * P],
                                         mo[:, mb + P:mb + 2 * P])
                    s = wpool.tile([128, P], bf16, tag="s", bufs=2)
                    nc.vector.tensor_add(s[:], g2[:, 0:P], g2[:, P:2 * P])
                    ot = wpool.tile([128, P], out_dt, tag="ot", name="ot",
                                    bufs=4)
                    nc.scalar.activation(ot[:], s[:], AF.Silu)
                    nc.sync.dma_start(outT[dt, :, j0:j0 + P], ot[:])
                elif it % 2 == 1:
                    # AB for this duo: me += mo (two 2P accums: the CCE
                    # path faults on runs wider than 2048 elements)
                    nc.gpsimd.dma_start(me[:, 0:2 * P], mo[:, 0:2 * P],
                                        accum_op=ALU.add)
                    nc.gpsimd.dma_start(me[:, 2 * P:4 * P], mo[:, 2 * P:4 * P],
                                        accum_op=ALU.add)
                    stageC.append((me, duo_locs))
            while stageC:
                emit_c()
            while stageH:
                emit_silu()
    nc.compile()
    return nc


def _prep_shards(x, w1, w2, b2):
    """Host-side shard prep. Returns list of per-core in_maps."""
    import ml_dtypes
    bf16 = ml_dtypes.bfloat16

    n_dt, n_hc, tok = N_DT, N_HC, TOK
    b, t, d = x.shape
    shards_per_batch = (b * t // tok) // b

    w1_r = np.ascontiguousarray(
        w1.reshape(n_dt, 128, H).transpose(1, 0, 2).reshape(128, n_dt * H)
    ).astype(bf16)
    w2_r = np.ascontiguousarray(
        w2.reshape(n_hc, 128, n_dt, 128, W).transpose(2, 1, 0, 4, 3)
        .reshape(n_dt, 128, n_hc * W * 128)).astype(bf16)
    b2_r = np.ascontiguousarray(
        b2.reshape(n_dt, 128, W).transpose(1, 0, 2).reshape(128, n_dt * W)
    ).astype(np.float32)

    in_maps = []
    for core in range(N_CORES):
        bi, half = divmod(core, shards_per_batch)
        t0 = half * tok
        xh = np.zeros((tok + HALO, d), np.float32)
        lo = max(t0 - HALO, 0)
        xh[HALO - (t0 - lo):] = x[bi, lo: t0 + tok]
        xTc = np.zeros((n_dt, 128, XSTRIDE), bf16)
        xTc[:, :, : tok + HALO] = (
            xh.T.astype(bf16).reshape(n_dt, 128, tok + HALO))
        in_maps.append({"xT": xTc, "w1r": w1_r, "w2r": w2_r, "b2r": b2_r})
    return in_maps


_NC_CACHE = {}


def kernel(x, w1, w2, b2, trace=False):
    from concourse.bass_utils import run_bass_kernel_spmd

    mode = os.environ.get("KMODE", "v2")
    if mode not in _NC_CACHE:
        _NC_CACHE[mode] = _build_nc(mode=mode, out_f32=False)
    nc = _NC_CACHE[mode]

    in_maps = _prep_shards(
        np.asarray(x, np.float32), np.asarray(w1, np.float32),
        np.asarray(w2, np.float32), np.asarray(b2, np.float32))

    res = run_bass_kernel_spmd(nc, in_maps, core_ids=list(range(N_CORES)),
                               trace=trace)
    kernel.last_result = res

    shards_per_batch = (B * T // TOK) // B
    out = np.empty((B, T, D), np.float32)
    for core in range(N_CORES):
        bi, half = divmod(core, shards_per_batch)
        oT = res.results[core]["outT"]  # [n_dt, 128, tok]
        out[bi, half * TOK:(half + 1) * TOK] = (
            oT.reshape(D, TOK).T.astype(np.float32))
    return out


# revision 24
# speedup vs baseline: 1.1357x; 1.0174x over previous
"""Trainium2 Bass kernel for nn_DynamicShortConvolution.

Reference computation (per token t, channel d):
    h    = silu(x @ w1)                       # [T, H]
    flat = h @ w2 + b2                        # [T, D*W]
    k    = flat.reshape(T, D, W)
    out[t, d] = silu(sum_w k[t, d, w] * x[t - (W-1) + w, d])

Sharding: 8 cores, each one (batch, half-of-T) shard of 2048 tokens plus a
3-token left halo.  Per-core tensors are TRANSPOSED ([D, T], channels on
SBUF partitions) so the conv's token shift is a free-dim offset and both
matmuls consume/produce the natural layouts (no on-device transposes).

Schedule (engineered from the measured TRN2 cost model):
 - PE warm-up matmuls at t=0 cover the initial DMA latency (HAM stays warm).
 - mm1 accumulates dt-OUTER so each x d-tile is consumed as its DMA lands;
   x tiles stream on the sync HWDGE ring in consumption order, w2 follows.
 - mm2 evac per (dt, 1024-token pair): DVE does the two odd-tap fused
   (k+b)*x stt ops (PSUM 1x mode) plus the two even-tap bf16 2x-mode
   multiplies; ACT does the two even-tap biased PSUM evacs + final silu;
   the 3 tap-sum adds run as SBUF->SBUF accumulate-DMAs (SWDGE CCE) in
   mode 'v2', or on GpSimd/DVE in mode 'v1'.
"""

import os

import numpy as np

# Problem constants (hardcoded per harness contract).
B, T, D, H, W = 4, 4096, 2048, 256, 4
HALO = W - 1
N_CORES = 8
TOK = (B * T) // N_CORES  # tokens per core = 2048
N_DT = D // 128           # 16 d-tiles
N_HC = H // 128           # 2 h-tiles
XSTRIDE = TOK + HALO + 1  # 2052, even keeps per-dtile 4B alignment
CH = 512                  # mm1 token chunk (one PSUM bank of fp32)
P = 1024                  # mm2 token pair width (two banks per tap)
N_WARM = 14               # PE warm-up matmuls (~6us at cold clock)


def _build_nc(mode="v2", out_f32=False):
    import concourse.bass as bass  # noqa: F401
    import concourse.bacc as bacc
    import concourse.mybir as mybir
    import concourse.tile as tile
    from concourse.ap import AP as BassAP

    f32 = mybir.dt.float32
    bf16 = mybir.dt.bfloat16
    AF = mybir.ActivationFunctionType
    ALU = mybir.AluOpType

    tok, h = TOK, H
    n_dt, n_hc = N_DT, N_HC

    nc = bacc.Bacc()

    # DRAM I/O (host-prepared layouts; see _prep_shards)
    xT = nc.declare_dram_parameter("xT", [n_dt, 128, XSTRIDE], bf16, isOutput=False)
    w1r = nc.declare_dram_parameter("w1r", [128, n_dt * h], bf16, isOutput=False)
    # w2r[dt, p, hc*512 + w*128 + i] = w2[hc*128+p, (dt*128+i)*W + w]
    w2r = nc.declare_dram_parameter("w2r", [n_dt, 128, n_hc * W * 128], bf16,
                                    isOutput=False)
    # b2r[p, dt*W + w] = b2[(dt*128+p)*W + w]
    b2r = nc.declare_dram_parameter("b2r", [128, n_dt * W], f32, isOutput=False)
    out_dt = f32 if out_f32 else bf16
    outT = nc.declare_dram_parameter("outT", [n_dt, 128, tok], out_dt, isOutput=True)

    with tile.TileContext(nc) as tc:
        with (
            tc.tile_pool(name="resident", bufs=1) as rpool,
            tc.tile_pool(name="work", bufs=3) as wpool,
            tc.tile_pool(name="psum", bufs=4, space="PSUM") as ppool,
        ):
            # ---- PE warm-up (covers initial DMA latency, warms HAM) ----
            warm_sb = rpool.tile([128, 640], bf16, tag="warm")
            nc.vector.memset(warm_sb[:], 0.0)
            for _ in range(N_WARM):
                wt = ppool.tile([128, P], f32, tag="ps")
                nc.tensor.matmul(wt[:, :CH], warm_sb[:, :128],
                                 warm_sb[:, 128:640], start=True, stop=True)

            # ---- resident tiles ----
            xT_sb = rpool.tile([128, n_dt * XSTRIDE], bf16, tag="xT")
            w1_sb = rpool.tile([128, n_dt * h], bf16, tag="w1")
            w2_sb = rpool.tile([128, n_dt * n_hc * W * 128], bf16, tag="w2")
            b2_sb = rpool.tile([128, n_dt * W], f32, tag="b2")
            hT_sb = rpool.tile([128, n_hc * tok], bf16, tag="hT")

            # DMA order = consumption order: w1, then x d-tiles, then w2.
            # b2 rides the scalar (ACT) HWDGE ring so it never queues x.
            nc.scalar.dma_start(b2_sb[:], b2r[:])
            nc.sync.dma_start(w1_sb[:], w1r[:])
            for dt in range(n_dt):
                nc.sync.dma_start(
                    xT_sb[:, dt * XSTRIDE:(dt + 1) * XSTRIDE], xT[dt])
            for dt in range(n_dt):
                nc.sync.dma_start(
                    w2_sb[:, dt * 1024:(dt + 1) * 1024], w2r[dt])

            def x_slice(dt, col, n):
                return xT_sb[:, dt * XSTRIDE + col: dt * XSTRIDE + col + n]

            # ---- mm1: hT = silu(w1.T @ xT), dt-OUTER accumulation ----
            # 8 groups (hc, tc) live in 4 [128,1024] psum tiles (2 banks
            # each, one bank per group) so each arriving x d-tile is
            # consumed immediately.
            ph = [ppool.tile([128, P], f32, tag="ps", name=f"ph{i}")
                  for i in range(4)]

            def ph_half(g):
                c = (g % 2) * CH
                return ph[g // 2][:, c:c + CH]

            for dt in range(n_dt):
                for hc in range(n_hc):
                    for tcb in range(4):
                        g = hc * 4 + tcb
                        nc.tensor.matmul(
                            ph_half(g),
                            w1_sb[:, dt * h + hc * 128: dt * h + hc * 128 + 128],
                            x_slice(dt, HALO + tcb * CH, CH),
                            start=(dt == 0), stop=(dt == n_dt - 1))
            for g in range(8):
                hc, tcb = g // 4, g % 4
                nc.scalar.activation(
                    hT_sb[:, hc * tok + tcb * CH: hc * tok + (tcb + 1) * CH],
                    ph_half(g), AF.Silu)

            # ---- mm2 + conv + silu, per (dt, 1024-token pair) ----
            # Two iterations form a "duo" with tap sums split across two
            # tiles:  me = [e.m0 | e.m2 | o.m0 | o.m2]   (survives to silu)
            #         mo = [e.m1 | e.m3 | o.m1 | o.m3]   (freed after AB)
            # Tap-sum = two SBUF->SBUF accumulate-DMAs per duo (SWDGE CCE,
            # half the descriptor-gen of per-iteration DMAs — desc-gen on
            # GpSimd contends with DVE SBUF reads):
            #   AB: me += mo (contiguous 4P)   C: me.[m0s] += me.[m2s]
            # C lags two iterations, silu + out-DMA lag two more.
            stageC = []   # [(me, [(dt, j0), (dt, j0)])] awaiting C
            stageH = []   # [(me, half, dt, j0)] C emitted, awaiting silu

            def emit_c():
                me, locs = stageC.pop(0)
                sl = me[:, 0:P]
                dst = BassAP(tensor=sl.tensor, offset=sl.offset,
                             ap=[list(sl.ap[0]), [2 * P, 2], [1, P]])
                sl2 = me[:, P:2 * P]
                src = BassAP(tensor=sl2.tensor, offset=sl2.offset,
                             ap=[list(sl2.ap[0]), [2 * P, 2], [1, P]])
                nc.gpsimd.dma_start(dst, src, accum_op=ALU.add)
                for half, (pdt, pj0) in enumerate(locs):
                    stageH.append((me, half, pdt, pj0))

            def emit_silu():
                me, half, pdt, pj0 = stageH.pop(0)
                ot = wpool.tile([128, P], out_dt, tag="ot", name="ot", bufs=4)
                nc.scalar.activation(
                    ot[:], me[:, half * 2 * P: half * 2 * P + P], AF.Silu)
                nc.sync.dma_start(outT[pdt, :, pj0:pj0 + P], ot[:])

            me, mo, duo_locs = None, None, []
            for it in range(n_dt * (tok // P)):
                dt, pi = it // 2, it % 2
                j0 = pi * P
                kw = [ppool.tile([128, P], f32, tag="ps", name=f"kw{w}")
                      for w in range(W)]
                for w in range(W):
                    wcol = dt * (n_hc * W * 128) + w * 128
                    for hc in range(n_hc):
                        for tcj in range(2):
                            nc.tensor.matmul(
                                kw[w][:, tcj * CH:(tcj + 1) * CH],
                                w2_sb[:, wcol + hc * W * 128:
                                      wcol + hc * W * 128 + 128],
                                hT_sb[:, hc * tok + j0 + tcj * CH:
                                      hc * tok + j0 + (tcj + 1) * CH],
                                start=(hc == 0), stop=(hc == n_hc - 1))
                if it % 2 == 0:
                    me = wpool.tile([128, 4 * P], bf16, tag="me", bufs=4)
                    mo = wpool.tile([128, 4 * P], bf16, tag="mo", bufs=2)
                    duo_locs = []
                    if len(stageC) >= 2:
                        emit_c()  # C for the duo before last, deps met
                mb = (it % 2) * 2 * P
                duo_locs.append((dt, j0))
                bias = [b2_sb[:, dt * W + w: dt * W + w + 1]
                        for w in range(W)]
                kb = wpool.tile([128, 2 * P], bf16, tag="kb", bufs=4)
                # odd taps (2B-misaligned x windows): DVE fused
                # evac+bias+mul, PSUM 1x mode.  even taps: ACT biased
                # evac, then DVE bf16 2x-mode mul (x windows 4B-aligned)
                nc.vector.scalar_tensor_tensor(
                    mo[:, mb:mb + P], kw[1][:], bias[1],
                    x_slice(dt, j0 + 1, P), op0=ALU.add, op1=ALU.mult)
                nc.scalar.activation(kb[:, 0:P], kw[0][:], AF.Identity,
                                     bias=bias[0])
                nc.vector.tensor_mul(me[:, mb:mb + P], kb[:, 0:P],
                                     x_slice(dt, j0 + 0, P))
                nc.vector.scalar_tensor_tensor(
                    mo[:, mb + P:mb + 2 * P], kw[3][:], bias[3],
                    x_slice(dt, j0 + 3, P), op0=ALU.add, op1=ALU.mult)
                nc.scalar.activation(kb[:, P:2 * P], kw[2][:], AF.Identity,
                                     bias=bias[2])
                nc.vector.tensor_mul(me[:, mb + P:mb + 2 * P],
                                     kb[:, P:2 * P], x_slice(dt, j0 + 2, P))
                if len(stageH) >= 4:
                    emit_silu()
                n_it = n_dt * (tok // P)
                if it >= n_it - 4:
                    # tail iterations: engines drain soon, so sum the taps
                    # inline on DVE/ACT instead of the long DMA-accum chain
                    g2 = wpool.tile([128, 2 * P], bf16, tag="g2", bufs=2)
                    nc.vector.tensor_add(g2[:, 0:P], me[:, mb:mb + P],
                                         mo[:, mb:mb + P])
                    nc.vector.tensor_add(g2[:, P:2 * P],
                                         me[:, mb + P:mb + 2 * P],
                                         mo[:, mb + P:mb + 2 * P])
                    s = wpool.tile([128, P], bf16, tag="s", bufs=2)
                    nc.vector.tensor_add(s[:], g2[:, 0:P], g2[:, P:2 * P])
                    ot = wpool.tile([128, P], out_dt, tag="ot", name="ot",
                                    bufs=4)
                    nc.scalar.activation(ot[:], s[:], AF.Silu)
                    nc.sync.dma_start(outT[dt, :, j0:j0 + P], ot[:])
                elif it % 2 == 1:
                    # AB for this duo: me += mo (two 2P accums: the CCE
                    # path faults on runs wider than 2048 elements)
                    nc.gpsimd.dma_start(me[:, 0:2 * P], mo[:, 0:2 * P],
                                        accum_op=ALU.add)
                    nc.gpsimd.dma_start(me[:, 2 * P:4 * P], mo[:, 2 * P:4 * P],
                                        accum_op=ALU.add)
                    stageC.append((me, duo_locs))
            while stageC:
                emit_c()
            while stageH:
                emit_silu()
    nc.compile()
    return nc


def _prep_shards(x, w1, w2, b2):
    """Host-side shard prep. Returns list of per-core in_maps."""
    import ml_dtypes
    bf16 = ml_dtypes.bfloat16

    n_dt, n_hc, tok = N_DT, N_HC, TOK
    b, t, d = x.shape
    shards_per_batch = (b * t // tok) // b

    w1_r = np.ascontiguousarray(
        w1.reshape(n_dt, 128, H).transpose(1, 0, 2).reshape(128, n_dt * H)
    ).astype(bf16)
    w2_r = np.ascontiguousarray(
        w2.reshape(n_hc, 128, n_dt, 128, W).transpose(2, 1, 0, 4, 3)
        .reshape(n_dt, 128, n_hc * W * 128)).astype(bf16)
    b2_r = np.ascontiguousarray(
        b2.reshape(n_dt, 128, W).transpose(1, 0, 2).reshape(128, n_dt * W)
    ).astype(np.float32)

    in_maps = []
    for core in range(N_CORES):
        bi, half = divmod(core, shards_per_batch)
        t0 = half * tok
        xh = np.zeros((tok + HALO, d), np.float32)
        lo = max(t0 - HALO, 0)
        xh[HALO - (t0 - lo):] = x[bi, lo: t0 + tok]
        xTc = np.zeros((n_dt, 128, XSTRIDE), bf16)
        xTc[:, :, : tok + HALO] = (
            xh.T.astype(bf16).reshape(n_dt, 128, tok + HALO))
        in_maps.append({"xT": xTc, "w1r": w1_r, "w2r": w2_r, "b2r": b2_r})
    return in_maps


_NC_CACHE = {}


def kernel(x, w1, w2, b2, trace=False):
    from concourse.bass_utils import run_bass_kernel_spmd

    mode = os.environ.get("KMODE", "v2")
    if mode not in _NC_CACHE:
        _NC_CACHE[mode] = _build_nc(mode=mode, out_f32=False)
    nc = _NC_CACHE[mode]

    in_maps = _prep_shards(
        np.asarray(x, np.float32), np.asarray(w1, np.float32),
        np.asarray(w2, np.float32), np.asarray(b2, np.float32))

    res = run_bass_kernel_spmd(nc, in_maps, core_ids=list(range(N_CORES)),
                               trace=trace)
    kernel.last_result = res

    shards_per_batch = (B * T // TOK) // B
    out = np.empty((B, T, D), np.float32)
    for core in range(N_CORES):
        bi, half = divmod(core, shards_per_batch)
        oT = res.results[core]["outT"]  # [n_dt, 128, tok]
        out[bi, half * TOK:(half + 1) * TOK] = (
            oT.reshape(D, TOK).T.astype(np.float32))
    return out


# revision 25
# speedup vs baseline: 1.1380x; 1.0021x over previous
"""Trainium2 Bass kernel for nn_DynamicShortConvolution.

Reference computation (per token t, channel d):
    h    = silu(x @ w1)                       # [T, H]
    flat = h @ w2 + b2                        # [T, D*W]
    k    = flat.reshape(T, D, W)
    out[t, d] = silu(sum_w k[t, d, w] * x[t - (W-1) + w, d])

Sharding: 8 cores, each one (batch, half-of-T) shard of 2048 tokens plus a
3-token left halo.  Per-core tensors are TRANSPOSED ([D, T], channels on
SBUF partitions) so the conv's token shift is a free-dim offset and both
matmuls consume/produce the natural layouts (no on-device transposes).

Schedule (engineered from the measured TRN2 cost model):
 - PE warm-up matmuls at t=0 cover the initial DMA latency (HAM stays warm).
 - mm1 accumulates dt-OUTER so each x d-tile is consumed as its DMA lands;
   x tiles stream on the sync HWDGE ring in consumption order, w2 follows.
 - mm2 evac per (dt, 1024-token pair): DVE does the two odd-tap fused
   (k+b)*x stt ops (PSUM 1x mode) plus the two even-tap bf16 2x-mode
   multiplies; ACT does the two even-tap biased PSUM evacs + final silu;
   the 3 tap-sum adds run as SBUF->SBUF accumulate-DMAs (SWDGE CCE) in
   mode 'v2', or on GpSimd/DVE in mode 'v1'.
"""

import os

import numpy as np

# Problem constants (hardcoded per harness contract).
B, T, D, H, W = 4, 4096, 2048, 256, 4
HALO = W - 1
N_CORES = 8
TOK = (B * T) // N_CORES  # tokens per core = 2048
N_DT = D // 128           # 16 d-tiles
N_HC = H // 128           # 2 h-tiles
XSTRIDE = TOK + HALO + 1  # 2052, even keeps per-dtile 4B alignment
CH = 512                  # mm1 token chunk (one PSUM bank of fp32)
P = 1024                  # mm2 token pair width (two banks per tap)
N_WARM = 14               # PE warm-up matmuls (~6us at cold clock)


def _build_nc(mode="v2", out_f32=False):
    import concourse.bass as bass  # noqa: F401
    import concourse.bacc as bacc
    import concourse.mybir as mybir
    import concourse.tile as tile
    from concourse.ap import AP as BassAP

    f32 = mybir.dt.float32
    bf16 = mybir.dt.bfloat16
    AF = mybir.ActivationFunctionType
    ALU = mybir.AluOpType

    tok, h = TOK, H
    n_dt, n_hc = N_DT, N_HC

    nc = bacc.Bacc()

    # DRAM I/O (host-prepared layouts; see _prep_shards)
    xT = nc.declare_dram_parameter("xT", [n_dt, 128, XSTRIDE], bf16, isOutput=False)
    w1r = nc.declare_dram_parameter("w1r", [128, n_dt * h], bf16, isOutput=False)
    # w2r[dt, p, hc*512 + w*128 + i] = w2[hc*128+p, (dt*128+i)*W + w]
    w2r = nc.declare_dram_parameter("w2r", [n_dt, 128, n_hc * W * 128], bf16,
                                    isOutput=False)
    # b2r[p, dt*W + w] = b2[(dt*128+p)*W + w]
    b2r = nc.declare_dram_parameter("b2r", [128, n_dt * W], f32, isOutput=False)
    out_dt = f32 if out_f32 else bf16
    outT = nc.declare_dram_parameter("outT", [n_dt, 128, tok], out_dt, isOutput=True)

    with tile.TileContext(nc) as tc:
        with (
            tc.tile_pool(name="resident", bufs=1) as rpool,
            tc.tile_pool(name="work", bufs=3) as wpool,
            tc.tile_pool(name="psum", bufs=4, space="PSUM") as ppool,
        ):
            # ---- PE warm-up (covers initial DMA latency, warms HAM) ----
            warm_sb = rpool.tile([128, 640], bf16, tag="warm")
            nc.vector.memset(warm_sb[:], 0.0)
            for _ in range(N_WARM):
                wt = ppool.tile([128, P], f32, tag="ps")
                nc.tensor.matmul(wt[:, :CH], warm_sb[:, :128],
                                 warm_sb[:, 128:640], start=True, stop=True)

            # ---- resident tiles ----
            xT_sb = rpool.tile([128, n_dt * XSTRIDE], bf16, tag="xT")
            w1_sb = rpool.tile([128, n_dt * h], bf16, tag="w1")
            w2_sb = rpool.tile([128, n_dt * n_hc * W * 128], bf16, tag="w2")
            b2_sb = rpool.tile([128, n_dt * W], f32, tag="b2")
            hT_sb = rpool.tile([128, n_hc * tok], bf16, tag="hT")

            # DMA order = consumption order: w1, then x d-tiles, then w2.
            # b2 rides the scalar (ACT) HWDGE ring so it never queues x.
            nc.scalar.dma_start(b2_sb[:], b2r[:])
            nc.sync.dma_start(w1_sb[:], w1r[:])
            for dt in range(n_dt):
                nc.sync.dma_start(
                    xT_sb[:, dt * XSTRIDE:(dt + 1) * XSTRIDE], xT[dt])
            for dt in range(n_dt):
                nc.sync.dma_start(
                    w2_sb[:, dt * 1024:(dt + 1) * 1024], w2r[dt])

            def x_slice(dt, col, n):
                return xT_sb[:, dt * XSTRIDE + col: dt * XSTRIDE + col + n]

            # ---- mm1: hT = silu(w1.T @ xT), dt-OUTER accumulation ----
            # 8 groups (hc, tc) live in 4 [128,1024] psum tiles (2 banks
            # each, one bank per group) so each arriving x d-tile is
            # consumed immediately.
            ph = [ppool.tile([128, P], f32, tag="ps", name=f"ph{i}")
                  for i in range(4)]

            def ph_half(g):
                c = (g % 2) * CH
                return ph[g // 2][:, c:c + CH]

            for dt in range(n_dt):
                for hc in range(n_hc):
                    for tcb in range(4):
                        g = hc * 4 + tcb
                        nc.tensor.matmul(
                            ph_half(g),
                            w1_sb[:, dt * h + hc * 128: dt * h + hc * 128 + 128],
                            x_slice(dt, HALO + tcb * CH, CH),
                            start=(dt == 0), stop=(dt == n_dt - 1))
            for g in range(8):
                hc, tcb = g // 4, g % 4
                nc.scalar.activation(
                    hT_sb[:, hc * tok + tcb * CH: hc * tok + (tcb + 1) * CH],
                    ph_half(g), AF.Silu)

            # ---- mm2 + conv + silu, per (dt, 1024-token pair) ----
            # Two iterations form a "duo" with tap sums split across two
            # tiles:  me = [e.m0 | e.m2 | o.m0 | o.m2]   (survives to silu)
            #         mo = [e.m1 | e.m3 | o.m1 | o.m3]   (freed after AB)
            # Tap-sum = two SBUF->SBUF accumulate-DMAs per duo (SWDGE CCE,
            # half the descriptor-gen of per-iteration DMAs — desc-gen on
            # GpSimd contends with DVE SBUF reads):
            #   AB: me += mo (contiguous 4P)   C: me.[m0s] += me.[m2s]
            # C lags two iterations, silu + out-DMA lag two more.
            stageC = []   # [(me, [(dt, j0), (dt, j0)])] awaiting C
            stageH = []   # [(me, half, dt, j0)] C emitted, awaiting silu

            def emit_c():
                me, locs = stageC.pop(0)
                nc.gpsimd.dma_start(me[:, 0:P], me[:, P:2 * P],
                                    accum_op=ALU.add)
                nc.gpsimd.dma_start(me[:, 2 * P:3 * P], me[:, 3 * P:4 * P],
                                    accum_op=ALU.add)
                for half, (pdt, pj0) in enumerate(locs):
                    stageH.append((me, half, pdt, pj0))

            def emit_silu():
                me, half, pdt, pj0 = stageH.pop(0)
                ot = wpool.tile([128, P], out_dt, tag="ot", name="ot", bufs=4)
                nc.scalar.activation(
                    ot[:], me[:, half * 2 * P: half * 2 * P + P], AF.Silu)
                nc.sync.dma_start(outT[pdt, :, pj0:pj0 + P], ot[:])

            me, mo, duo_locs = None, None, []
            for it in range(n_dt * (tok // P)):
                dt, pi = it // 2, it % 2
                j0 = pi * P
                kw = [ppool.tile([128, P], f32, tag="ps", name=f"kw{w}")
                      for w in range(W)]
                for w in range(W):
                    wcol = dt * (n_hc * W * 128) + w * 128
                    for hc in range(n_hc):
                        for tcj in range(2):
                            nc.tensor.matmul(
                                kw[w][:, tcj * CH:(tcj + 1) * CH],
                                w2_sb[:, wcol + hc * W * 128:
                                      wcol + hc * W * 128 + 128],
                                hT_sb[:, hc * tok + j0 + tcj * CH:
                                      hc * tok + j0 + (tcj + 1) * CH],
                                start=(hc == 0), stop=(hc == n_hc - 1))
                if it % 2 == 0:
                    me = wpool.tile([128, 4 * P], bf16, tag="me", bufs=5)
                    mo = wpool.tile([128, 4 * P], bf16, tag="mo", bufs=2)
                    duo_locs = []
                    if len(stageC) >= 2:
                        emit_c()  # C for the duo before last, deps met
                mb = (it % 2) * 2 * P
                duo_locs.append((dt, j0))
                bias = [b2_sb[:, dt * W + w: dt * W + w + 1]
                        for w in range(W)]
                kb = wpool.tile([128, 2 * P], bf16, tag="kb", bufs=4)
                # odd taps (2B-misaligned x windows): DVE fused
                # evac+bias+mul, PSUM 1x mode.  even taps: ACT biased
                # evac, then DVE bf16 2x-mode mul (x windows 4B-aligned)
                nc.vector.scalar_tensor_tensor(
                    mo[:, mb:mb + P], kw[1][:], bias[1],
                    x_slice(dt, j0 + 1, P), op0=ALU.add, op1=ALU.mult)
                nc.scalar.activation(kb[:, 0:P], kw[0][:], AF.Identity,
                                     bias=bias[0])
                nc.vector.tensor_mul(me[:, mb:mb + P], kb[:, 0:P],
                                     x_slice(dt, j0 + 0, P))
                nc.vector.scalar_tensor_tensor(
                    mo[:, mb + P:mb + 2 * P], kw[3][:], bias[3],
                    x_slice(dt, j0 + 3, P), op0=ALU.add, op1=ALU.mult)
                nc.scalar.activation(kb[:, P:2 * P], kw[2][:], AF.Identity,
                                     bias=bias[2])
                nc.vector.tensor_mul(me[:, mb + P:mb + 2 * P],
                                     kb[:, P:2 * P], x_slice(dt, j0 + 2, P))
                if len(stageH) >= 6:
                    emit_silu()
                n_it = n_dt * (tok // P)
                if it >= n_it - 4:
                    # tail iterations: engines drain soon, so sum the taps
                    # inline on DVE/ACT instead of the long DMA-accum chain
                    g2 = wpool.tile([128, 2 * P], bf16, tag="g2", bufs=2)
                    nc.vector.tensor_add(g2[:, 0:P], me[:, mb:mb + P],
                                         mo[:, mb:mb + P])
                    nc.vector.tensor_add(g2[:, P:2 * P],
                                         me[:, mb + P:mb + 2 * P],
                                         mo[:, mb + P:mb + 2 * P])
                    s = wpool.tile([128, P], bf16, tag="s", bufs=2)
                    nc.vector.tensor_add(s[:], g2[:, 0:P], g2[:, P:2 * P])
                    ot = wpool.tile([128, P], out_dt, tag="ot", name="ot",
                                    bufs=4)
                    nc.scalar.activation(ot[:], s[:], AF.Silu)
                    nc.sync.dma_start(outT[dt, :, j0:j0 + P], ot[:])
                elif it % 2 == 1:
                    # AB for this duo: me += mo (two 2P accums: the CCE
                    # path faults on runs wider than 2048 elements)
                    nc.gpsimd.dma_start(me[:, 0:2 * P], mo[:, 0:2 * P],
                                        accum_op=ALU.add)
                    nc.gpsimd.dma_start(me[:, 2 * P:4 * P], mo[:, 2 * P:4 * P],
                                        accum_op=ALU.add)
                    stageC.append((me, duo_locs))
            while stageC:
                emit_c()
            while stageH:
                emit_silu()
    nc.compile()
    return nc


def _prep_shards(x, w1, w2, b2):
    """Host-side shard prep. Returns list of per-core in_maps."""
    import ml_dtypes
    bf16 = ml_dtypes.bfloat16

    n_dt, n_hc, tok = N_DT, N_HC, TOK
    b, t, d = x.shape
    shards_per_batch = (b * t // tok) // b

    w1_r = np.ascontiguousarray(
        w1.reshape(n_dt, 128, H).transpose(1, 0, 2).reshape(128, n_dt * H)
    ).astype(bf16)
    w2_r = np.ascontiguousarray(
        w2.reshape(n_hc, 128, n_dt, 128, W).transpose(2, 1, 0, 4, 3)
        .reshape(n_dt, 128, n_hc * W * 128)).astype(bf16)
    b2_r = np.ascontiguousarray(
        b2.reshape(n_dt, 128, W).transpose(1, 0, 2).reshape(128, n_dt * W)
    ).astype(np.float32)

    in_maps = []
    for core in range(N_CORES):
        bi, half = divmod(core, shards_per_batch)
        t0 = half * tok
        xh = np.zeros((tok + HALO, d), np.float32)
        lo = max(t0 - HALO, 0)
        xh[HALO - (t0 - lo):] = x[bi, lo: t0 + tok]
        xTc = np.zeros((n_dt, 128, XSTRIDE), bf16)
        xTc[:, :, : tok + HALO] = (
            xh.T.astype(bf16).reshape(n_dt, 128, tok + HALO))
        in_maps.append({"xT": xTc, "w1r": w1_r, "w2r": w2_r, "b2r": b2_r})
    return in_maps


_NC_CACHE = {}


def kernel(x, w1, w2, b2, trace=False):
    from concourse.bass_utils import run_bass_kernel_spmd

    mode = os.environ.get("KMODE", "v2")
    if mode not in _NC_CACHE:
        _NC_CACHE[mode] = _build_nc(mode=mode, out_f32=False)
    nc = _NC_CACHE[mode]

    in_maps = _prep_shards(
        np.asarray(x, np.float32), np.asarray(w1, np.float32),
        np.asarray(w2, np.float32), np.asarray(b2, np.float32))

    res = run_bass_kernel_spmd(nc, in_maps, core_ids=list(range(N_CORES)),
                               trace=trace)
    kernel.last_result = res

    shards_per_batch = (B * T // TOK) // B
    out = np.empty((B, T, D), np.float32)
    for core in range(N_CORES):
        bi, half = divmod(core, shards_per_batch)
        oT = res.results[core]["outT"]  # [n_dt, 128, tok]
        out[bi, half * TOK:(half + 1) * TOK] = (
            oT.reshape(D, TOK).T.astype(np.float32))
    return out


# revision 27
# speedup vs baseline: 1.1508x; 1.0112x over previous
"""Trainium2 Bass kernel for nn_DynamicShortConvolution.

Reference computation (per token t, channel d):
    h    = silu(x @ w1)                       # [T, H]
    flat = h @ w2 + b2                        # [T, D*W]
    k    = flat.reshape(T, D, W)
    out[t, d] = silu(sum_w k[t, d, w] * x[t - (W-1) + w, d])

Sharding: 8 cores, each one (batch, half-of-T) shard of 2048 tokens plus a
3-token left halo.  Per-core tensors are TRANSPOSED ([D, T], channels on
SBUF partitions) so the conv's token shift is a free-dim offset and both
matmuls consume/produce the natural layouts (no on-device transposes).

Schedule (engineered from the measured TRN2 cost model):
 - PE warm-up matmuls at t=0 cover the initial DMA latency (HAM stays warm).
 - mm1 accumulates dt-OUTER so each x d-tile is consumed as its DMA lands;
   x tiles stream on the sync HWDGE ring in consumption order, w2 follows.
 - mm2 evac per (dt, 1024-token pair): DVE does the two odd-tap fused
   (k+b)*x stt ops (PSUM 1x mode) plus the two even-tap bf16 2x-mode
   multiplies; ACT does the two even-tap biased PSUM evacs + final silu.
 - The 3 tap-sum adds per iteration run as SBUF->SBUF accumulate-DMAs
   (SWDGE CCE add), batched over iteration duos and software-pipelined
   several iterations deep so no engine FIFO ever blocks on a DMA
   completion (GpSimd tensor ops are avoided entirely: their SBUF port
   wars with DVE 2-tensor ops measured 5x slowdowns).
 - The last four iterations sum taps inline on DVE instead, trimming the
   serial DMA-chain drain off the kernel tail.
"""

import numpy as np

# Problem constants (hardcoded per harness contract).
B, T, D, H, W = 4, 4096, 2048, 256, 4
HALO = W - 1
N_CORES = 8
TOK = (B * T) // N_CORES  # tokens per core = 2048
N_DT = D // 128           # 16 d-tiles
N_HC = H // 128           # 2 h-tiles
XSTRIDE = TOK + HALO + 1  # 2052, even keeps per-dtile 4B alignment
CH = 512                  # mm1 token chunk (one PSUM bank of fp32)
P = 1024                  # mm2 token pair width (two banks per tap)
N_WARM = 14               # PE warm-up matmuls (~6us at cold clock)


def _build_nc(out_f32=False):
    import concourse.bass as bass  # noqa: F401
    import concourse.bacc as bacc
    import concourse.mybir as mybir
    import concourse.tile as tile
    from concourse.ap import AP as BassAP

    f32 = mybir.dt.float32
    bf16 = mybir.dt.bfloat16
    AF = mybir.ActivationFunctionType
    ALU = mybir.AluOpType

    tok, h = TOK, H
    n_dt, n_hc = N_DT, N_HC

    nc = bacc.Bacc()

    # DRAM I/O (host-prepared layouts; see _prep_shards)
    xT = nc.declare_dram_parameter("xT", [n_dt, 128, XSTRIDE], bf16, isOutput=False)
    w1r = nc.declare_dram_parameter("w1r", [128, n_dt * h], bf16, isOutput=False)
    # w2r[dt, p, hc*512 + w*128 + i] = w2[hc*128+p, (dt*128+i)*W + w]
    w2r = nc.declare_dram_parameter("w2r", [n_dt, 128, n_hc * W * 128], bf16,
                                    isOutput=False)
    # b2r[p, dt*W + w] = b2[(dt*128+p)*W + w]
    b2r = nc.declare_dram_parameter("b2r", [128, n_dt * W], f32, isOutput=False)
    out_dt = f32 if out_f32 else bf16
    outT = nc.declare_dram_parameter("outT", [n_dt, 128, tok], out_dt, isOutput=True)

    with tile.TileContext(nc) as tc:
        with (
            tc.tile_pool(name="resident", bufs=1) as rpool,
            tc.tile_pool(name="work", bufs=3) as wpool,
            tc.tile_pool(name="psum", bufs=4, space="PSUM") as ppool,
        ):
            # ---- PE warm-up (covers initial DMA latency, warms HAM) ----
            warm_sb = rpool.tile([128, 640], bf16, tag="warm")
            nc.vector.memset(warm_sb[:], 0.0)
            for _ in range(N_WARM):
                wt = ppool.tile([128, P], f32, tag="ps")
                nc.tensor.matmul(wt[:, :CH], warm_sb[:, :128],
                                 warm_sb[:, 128:640], start=True, stop=True)

            # ---- resident tiles ----
            xT_sb = rpool.tile([128, n_dt * XSTRIDE], bf16, tag="xT")
            w1_sb = rpool.tile([128, n_dt * h], bf16, tag="w1")
            w2_sb = rpool.tile([128, n_dt * n_hc * W * 128], bf16, tag="w2")
            b2_sb = rpool.tile([128, n_dt * W], f32, tag="b2")
            hT_sb = rpool.tile([128, n_hc * tok], bf16, tag="hT")

            # DMA order = consumption order: w1, then x d-tiles, then w2.
            # b2 rides the scalar (ACT) HWDGE ring so it never queues x.
            nc.scalar.dma_start(b2_sb[:], b2r[:])
            nc.sync.dma_start(w1_sb[:], w1r[:])
            for dt in range(n_dt):
                nc.sync.dma_start(
                    xT_sb[:, dt * XSTRIDE:(dt + 1) * XSTRIDE], xT[dt])
            for dt in range(n_dt):
                nc.sync.dma_start(
                    w2_sb[:, dt * 1024:(dt + 1) * 1024], w2r[dt])

            def x_slice(dt, col, n):
                return xT_sb[:, dt * XSTRIDE + col: dt * XSTRIDE + col + n]

            # ---- mm1: hT = silu(w1.T @ xT), dt-OUTER accumulation ----
            # 8 groups (hc, tc) live in 4 [128,1024] psum tiles (2 banks
            # each, one bank per group) so each arriving x d-tile is
            # consumed immediately.
            ph = [ppool.tile([128, P], f32, tag="ps", name=f"ph{i}")
                  for i in range(4)]

            def ph_half(g):
                c = (g % 2) * CH
                return ph[g // 2][:, c:c + CH]

            for dt in range(n_dt):
                for hc in range(n_hc):
                    for tcb in range(4):
                        g = hc * 4 + tcb
                        nc.tensor.matmul(
                            ph_half(g),
                            w1_sb[:, dt * h + hc * 128: dt * h + hc * 128 + 128],
                            x_slice(dt, HALO + tcb * CH, CH),
                            start=(dt == 0), stop=(dt == n_dt - 1))
            for g in range(8):
                hc, tcb = g // 4, g % 4
                nc.scalar.activation(
                    hT_sb[:, hc * tok + tcb * CH: hc * tok + (tcb + 1) * CH],
                    ph_half(g), AF.Silu)

            # ---- mm2 + conv + silu, per (dt, 1024-token pair) ----
            # Two iterations form a "duo" with tap sums split across two
            # tiles:  me = [e.m0 | e.m2 | o.m0 | o.m2]   (survives to silu)
            #         mo = [e.m1 | e.m3 | o.m1 | o.m3]   (freed after AB)
            # Tap-sum = two SBUF->SBUF accumulate-DMAs per duo (SWDGE CCE,
            # half the descriptor-gen of per-iteration DMAs — desc-gen on
            # GpSimd contends with DVE SBUF reads):
            #   AB: me += mo (contiguous 4P)   C: me.[m0s] += me.[m2s]
            # C lags two iterations, silu + out-DMA lag two more.
            stageC = []   # [(me, [(dt, j0), (dt, j0)])] awaiting C
            stageH = []   # [(me, half, dt, j0)] C emitted, awaiting silu

            def emit_c():
                me, locs = stageC.pop(0)
                nc.gpsimd.dma_start(me[:, 0:P], me[:, P:2 * P],
                                    accum_op=ALU.add)
                nc.gpsimd.dma_start(me[:, 2 * P:3 * P], me[:, 3 * P:4 * P],
                                    accum_op=ALU.add)
                for half, (pdt, pj0) in enumerate(locs):
                    stageH.append((me, half, pdt, pj0))

            def emit_silu():
                me, half, pdt, pj0 = stageH.pop(0)
                ot = wpool.tile([128, P], out_dt, tag="ot", name="ot", bufs=4)
                nc.scalar.activation(
                    ot[:], me[:, half * 2 * P: half * 2 * P + P], AF.Silu)
                nc.sync.dma_start(outT[pdt, :, pj0:pj0 + P], ot[:])

            me, mo, duo_locs = None, None, []
            for it in range(n_dt * (tok // P)):
                dt, pi = it // 2, it % 2
                j0 = pi * P
                kw = [ppool.tile([128, P], f32, tag="ps", name=f"kw{w}")
                      for w in range(W)]
                for w in range(W):
                    wcol = dt * (n_hc * W * 128) + w * 128
                    for hc in range(n_hc):
                        for tcj in range(2):
                            nc.tensor.matmul(
                                kw[w][:, tcj * CH:(tcj + 1) * CH],
                                w2_sb[:, wcol + hc * W * 128:
                                      wcol + hc * W * 128 + 128],
                                hT_sb[:, hc * tok + j0 + tcj * CH:
                                      hc * tok + j0 + (tcj + 1) * CH],
                                start=(hc == 0), stop=(hc == n_hc - 1))
                if it % 2 == 0:
                    me = wpool.tile([128, 4 * P], bf16, tag="me", bufs=5)
                    mo = wpool.tile([128, 4 * P], bf16, tag="mo", bufs=2)
                    duo_locs = []
                    if len(stageC) >= 2:
                        emit_c()  # C for the duo before last, deps met
                mb = (it % 2) * 2 * P
                duo_locs.append((dt, j0))
                bias = [b2_sb[:, dt * W + w: dt * W + w + 1]
                        for w in range(W)]
                kb = wpool.tile([128, 2 * P], bf16, tag="kb", bufs=4)
                # odd taps (2B-misaligned x windows): DVE fused
                # evac+bias+mul, PSUM 1x mode.  even taps: ACT biased
                # evac, then DVE bf16 2x-mode mul (x windows 4B-aligned)
                nc.vector.scalar_tensor_tensor(
                    mo[:, mb:mb + P], kw[1][:], bias[1],
                    x_slice(dt, j0 + 1, P), op0=ALU.add, op1=ALU.mult)
                nc.scalar.activation(kb[:, 0:P], kw[0][:], AF.Identity,
                                     bias=bias[0])
                nc.vector.tensor_mul(me[:, mb:mb + P], kb[:, 0:P],
                                     x_slice(dt, j0 + 0, P))
                nc.vector.scalar_tensor_tensor(
                    mo[:, mb + P:mb + 2 * P], kw[3][:], bias[3],
                    x_slice(dt, j0 + 3, P), op0=ALU.add, op1=ALU.mult)
                nc.scalar.activation(kb[:, P:2 * P], kw[2][:], AF.Identity,
                                     bias=bias[2])
                nc.vector.tensor_mul(me[:, mb + P:mb + 2 * P],
                                     kb[:, P:2 * P], x_slice(dt, j0 + 2, P))
                if len(stageH) >= 6:
                    emit_silu()
                n_it = n_dt * (tok // P)
                if it >= n_it - 4:
                    # tail iterations: engines drain soon, so sum the taps
                    # inline on DVE/ACT instead of the long DMA-accum chain
                    g2 = wpool.tile([128, 2 * P], bf16, tag="g2", bufs=2)
                    nc.vector.tensor_add(g2[:, 0:P], me[:, mb:mb + P],
                                         mo[:, mb:mb + P])
                    nc.vector.tensor_add(g2[:, P:2 * P],
                                         me[:, mb + P:mb + 2 * P],
                                         mo[:, mb + P:mb + 2 * P])
                    s = wpool.tile([128, P], bf16, tag="s", bufs=2)
                    nc.vector.tensor_add(s[:], g2[:, 0:P], g2[:, P:2 * P])
                    ot = wpool.tile([128, P], out_dt, tag="ot", name="ot",
                                    bufs=4)
                    nc.scalar.activation(ot[:], s[:], AF.Silu)
                    nc.sync.dma_start(outT[dt, :, j0:j0 + P], ot[:])
                elif it % 2 == 1:
                    # AB for this duo: me += mo (two 2P accums: the CCE
                    # path faults on runs wider than 2048 elements)
                    nc.gpsimd.dma_start(me[:, 0:2 * P], mo[:, 0:2 * P],
                                        accum_op=ALU.add)
                    nc.gpsimd.dma_start(me[:, 2 * P:4 * P], mo[:, 2 * P:4 * P],
                                        accum_op=ALU.add)
                    stageC.append((me, duo_locs))
            while stageC:
                emit_c()
            while stageH:
                emit_silu()
    nc.compile()
    return nc


def _prep_shards(x, w1, w2, b2):
    """Host-side shard prep. Returns list of per-core in_maps."""
    import ml_dtypes
    bf16 = ml_dtypes.bfloat16

    n_dt, n_hc, tok = N_DT, N_HC, TOK
    b, t, d = x.shape
    shards_per_batch = (b * t // tok) // b

    w1_r = np.ascontiguousarray(
        w1.reshape(n_dt, 128, H).transpose(1, 0, 2).reshape(128, n_dt * H)
    ).astype(bf16)
    w2_r = np.ascontiguousarray(
        w2.reshape(n_hc, 128, n_dt, 128, W).transpose(2, 1, 0, 4, 3)
        .reshape(n_dt, 128, n_hc * W * 128)).astype(bf16)
    b2_r = np.ascontiguousarray(
        b2.reshape(n_dt, 128, W).transpose(1, 0, 2).reshape(128, n_dt * W)
    ).astype(np.float32)

    in_maps = []
    for core in range(N_CORES):
        bi, half = divmod(core, shards_per_batch)
        t0 = half * tok
        xh = np.zeros((tok + HALO, d), np.float32)
        lo = max(t0 - HALO, 0)
        xh[HALO - (t0 - lo):] = x[bi, lo: t0 + tok]
        xTc = np.zeros((n_dt, 128, XSTRIDE), bf16)
        xTc[:, :, : tok + HALO] = (
            xh.T.astype(bf16).reshape(n_dt, 128, tok + HALO))
        in_maps.append({"xT": xTc, "w1r": w1_r, "w2r": w2_r, "b2r": b2_r})
    return in_maps


_NC_CACHE = {}


def kernel(x, w1, w2, b2, trace=False):
    from concourse.bass_utils import run_bass_kernel_spmd

    if "nc" not in _NC_CACHE:
        _NC_CACHE["nc"] = _build_nc(out_f32=False)
    nc = _NC_CACHE["nc"]

    in_maps = _prep_shards(
        np.asarray(x, np.float32), np.asarray(w1, np.float32),
        np.asarray(w2, np.float32), np.asarray(b2, np.float32))

    res = run_bass_kernel_spmd(nc, in_maps, core_ids=list(range(N_CORES)),
                               trace=trace)
    kernel.last_result = res

    shards_per_batch = (B * T // TOK) // B
    out = np.empty((B, T, D), np.float32)
    for core in range(N_CORES):
        bi, half = divmod(core, shards_per_batch)
        oT = res.results[core]["outT"]  # [n_dt, 128, tok]
        out[bi, half * TOK:(half + 1) * TOK] = (
            oT.reshape(D, TOK).T.astype(np.float32))
    return out
